# revision 6
# baseline (speedup 1.0000x reference)
"""BankedLinear (MoE-style banked linear) Trainium2 Bass kernel, v2.

Math: out[n] = sum_k bank_weights[n,k] * (tensor[n] @ W[sel[n,k]] + bias[sel[n,k]])
Shapes: tensor [8192,128] f32, bank_weights [8192,2] f32, bank_selections [8192,2] int,
        weights [64,128,128] f32, bias [64,128] f32 -> out [8192,128] f32.

Strategy (data parallel over tokens, bank-pruned bf16 weights: fixed
ownership gives each core exactly 48 of the 64 banks, mapped into shared
"virtual" bank slots so the SPMD program stays identical across cores):
  - 8 cores x 1024 tokens. The host computes routing metadata and prepares
    each core's shard in the exact on-chip layout so the device does zero
    gathers/transposes on the input path:
      * each token's two (bank, weight) pairs are split into a primary (A)
        and secondary (B) slot space, each sorted by bank into contiguous
        per-bank segments (slot = t*128 + p grid);
      * xsT [128(in), CtotA+CtotB] bf16 holds the pre-gathered, pre-transposed
        token columns (pads zero);
      * W is uploaded once as [128(in), 64*128] bf16 (full-bandwidth layout);
      * S [64, Ctot] bf16 is the 0/1 bank-indicator used to fold the bias in
        via one accumulating matmul per psum group (psum += bias^T @ S).
  - On device per core:
      1. per-bank matmuls psum[:, seg] = W_b^T @ xsT[:, seg] (bf16 in, f32 acc)
         packed into <=512-wide psum groups; one bias matmul accumulates onto
         each full group; psum -> ysT bf16 copies split across DVE/ACT.
      2. PE-transpose each 128-slot block back to row layout, scaling rows by
         the per-slot bank_weights during the PSUM->SBUF copy -> yrows f32.
      3. A rows ARE the output rows (host picks the output row order), stored
         contiguously to out_d; B rows are dma_scatter_add-ed onto their
         token's A row (prepare_only descriptor gen early, trigger at the
         end), eliminating the DRAM y-scratch round trip entirely.
  - Host unshard: out[token] = out_d rows at the token's A slot.
"""

import numpy as np
import ml_dtypes

N, K, IN, OUT, NUM_BANKS = 8192, 2, 128, 128, 64
NCORES = 8
NLOC = N // NCORES  # tokens per core
P = 128
PSUM_FREE = 512  # fp32 columns per psum bank
BF16 = ml_dtypes.bfloat16
FP8 = ml_dtypes.float8_e4m3


def _routing_plan(sel_all):
    """Assign tokens to cores and pick a primary pair per token.

    Bank ownership is fixed combinatorially first: each bank excludes exactly
    2 of the 8 cores (all 28 pairs twice + an 8-cycle), so every core owns
    exactly 48 banks and any bank pair still has >=4 candidate cores. Each
    core then only uploads its own 48 banks, mapped into shared "virtual"
    bank slots (per-core banks sorted by load so the SPMD caps stay tight).

    Returns (assign [NCORES][NLOC] token ids, prim [N], order [NCORES][NV]
    physical bank per virtual slot, capsA, capsB) with caps over virtual
    slots."""
    import itertools
    sel_all = np.asarray(sel_all).astype(np.int64)
    pairs = list(itertools.combinations(range(NCORES), 2))
    excl = pairs + pairs + [(i, (i + 1) % NCORES) for i in range(NCORES)]
    owned = np.ones((NCORES, NUM_BANKS), dtype=bool)
    for b, (c1, c2) in enumerate(excl[:NUM_BANKS]):
        owned[c1, b] = False
        owned[c2, b] = False

    gcount = np.bincount(sel_all.reshape(-1), minlength=NUM_BANKS)
    ideal = -(-gcount // 12)  # ~6 owners x 2 spaces per bank

    cntA = np.zeros((NCORES, NUM_BANKS), dtype=np.int64)
    cntB = np.zeros((NCORES, NUM_BANKS), dtype=np.int64)
    fill = np.zeros(NCORES, dtype=np.int64)
    assign = [[] for _ in range(NCORES)]
    prim = np.zeros(N, dtype=np.int64)
    stuck = []
    for n in range(N):
        b0, b1 = int(sel_all[n, 0]), int(sel_all[n, 1])
        best = None
        bestkey = None
        for c in range(NCORES):
            if fill[c] >= NLOC:
                continue
            pen = 0 if (owned[c, b0] and owned[c, b1]) else 10000
            for k in (0, 1):
                bp, bs = (b0, b1) if k == 0 else (b1, b0)
                over = max(0, cntA[c, bp] + 1 - ideal[bp]) + max(
                    0, cntB[c, bs] + 1 - ideal[bs])
                key = (pen + over, cntA[c, bp] + cntB[c, bs], fill[c])
                if bestkey is None or key < bestkey:
                    bestkey, best = key, (c, k)
        c, k = best
        if not (owned[c, sel_all[n, 0]] and owned[c, sel_all[n, 1]]):
            stuck.append(n)
        bp, bs = (b0, b1) if k == 0 else (b1, b0)
        cntA[c, bp] += 1
        cntB[c, bs] += 1
        fill[c] += 1
        assign[c].append(n)
        prim[n] = k

    assign = np.array(assign, dtype=np.int64)
    core_of = np.zeros(N, dtype=np.int64)
    for c in range(NCORES):
        core_of[assign[c]] = c
    # repair: swap each stuck token with one from a core that owns its banks
    for n in stuck:
        c_bad = int(core_of[n])
        b0, b1 = int(sel_all[n, 0]), int(sel_all[n, 1])
        done = False
        for c_good in range(NCORES):
            if done or not (owned[c_good, b0] and owned[c_good, b1]):
                continue
            for n2 in assign[c_good]:
                if (owned[c_bad, sel_all[n2, 0]]
                        and owned[c_bad, sel_all[n2, 1]]):
                    i1 = int(np.nonzero(assign[c_bad] == n)[0][0])
                    i2 = int(np.nonzero(assign[c_good] == n2)[0][0])
                    assign[c_bad][i1], assign[c_good][i2] = n2, n
                    for (cc, nn, sgn) in ((c_bad, n, -1), (c_bad, n2, 1),
                                          (c_good, n2, -1), (c_good, n, 1)):
                        kp = int(prim[nn])
                        cntA[cc, sel_all[nn, kp]] += sgn
                        cntB[cc, sel_all[nn, 1 - kp]] += sgn
                    done = True
                    break
        assert done, "could not repair token->core ownership"

    # virtual slots: per core, banks sorted by total load
    NV = int(max(owned[c].sum() for c in range(NCORES)))
    NV = -(-NV // 8) * 8
    order = np.full((NCORES, NV), -1, dtype=np.int64)
    capsA = np.zeros(NV, dtype=np.int64)
    capsB = np.zeros(NV, dtype=np.int64)
    for c in range(NCORES):
        bl = np.nonzero(owned[c])[0]
        o = bl[np.argsort(-(cntA[c, bl] + cntB[c, bl]))]
        order[c, :len(o)] = o
        capsA[:len(o)] = np.maximum(capsA[:len(o)], cntA[c, o])
        capsB[:len(o)] = np.maximum(capsB[:len(o)], cntB[c, o])
    # pad each space's total to a multiple of 128 (transpose/store block grid)
    for caps in (capsA, capsB):
        pad = (-int(caps.sum())) % P
        for i in range(pad):
            caps[i % NV] += 1
    return assign, prim, order, capsA, capsB


def _wrap_idx(flat_idx):
    """Wrap a flat int16 index list into the [128, n//16] SWDGE layout:
    index i lives at [i % 16, i // 16], replicated across the 8 Q7 groups."""
    n = flat_idx.shape[0]
    assert n % 16 == 0
    w = flat_idx.reshape(n // 16, 16).T.astype(np.int16)
    return np.tile(w, (8, 1))


def _plan_groups(capsA, capsB, CtotA):
    """Segments (bank, col, width) over the concatenated A|B slot axis, packed
    into <=PSUM_FREE psum groups. Returns [(col0, width, [(bank, so, cb)])]."""
    chunks = []
    col = 0
    for caps, base in ((capsA, 0), (capsB, CtotA)):
        col = base
        for b in range(len(caps)):
            cb = int(caps[b])
            while cb > 0:
                w = min(cb, PSUM_FREE - (col % PSUM_FREE) if col % PSUM_FREE
                        else PSUM_FREE, PSUM_FREE)
                # cap chunk also at group boundary so groups stay aligned
                chunks.append((b, col, w))
                col += w
                cb -= w
    groups = []
    cur = None
    for (b, ob, cb) in chunks:
        if cur is not None and (ob + cb - cur[0]) <= PSUM_FREE:
            cur[2].append((b, ob - cur[0], cb))
            cur[1] = ob + cb - cur[0]
        else:
            if cur is not None:
                groups.append(tuple(cur))
            cur = [ob, cb, [(b, 0, cb)]]
    groups.append(tuple(cur))
    return groups


def _build_program(capsA, capsB):
    import concourse.bacc as bacc
    import concourse.tile as tile
    from concourse import mybir, library_config
    from concourse.masks import make_identity
    from concourse.tile import add_dep_helper

    f32 = mybir.dt.float32
    bf16 = mybir.dt.bfloat16
    fp8 = mybir.dt.float8e4
    i16 = mybir.dt.int16

    CtotA = int(capsA.sum())
    CtotB = int(capsB.sum())
    Ctot = CtotA + CtotB
    nblk = Ctot // P
    nblkA = CtotA // P
    nv = len(capsA)
    groups = _plan_groups(capsA, capsB, CtotA)

    nc = bacc.Bacc(None, target_bir_lowering=False, debug=False)

    xsT_d = nc.declare_dram_parameter("xst", [P, Ctot], bf16, isOutput=False)
    w_d = nc.declare_dram_parameter("wts", [P, nv * OUT], bf16,
                                    isOutput=False)
    s_d = nc.declare_dram_parameter("smat", [nv, Ctot], fp8,
                                    isOutput=False)
    bias_d = nc.declare_dram_parameter("biasb", [nv, OUT], fp8,
                                       isOutput=False)
    bws_d = nc.declare_dram_parameter("bws", [P, nblk], f32, isOutput=False)
    idx_d = nc.declare_dram_parameter("idxb", [P, CtotB // 16], i16,
                                      isOutput=False)
    # P extra dump rows receive the pad slots' zero scatter-adds: HW DMA
    # engines do not serialize concurrent RMWs to one row, so every real
    # target must be unique and pads must stay off the real rows
    out_d = nc.declare_dram_parameter("out", [CtotA + P, OUT], f32,
                                      isOutput=True)

    BPW = 8  # banks per weight upload tile
    NW = nv // BPW

    with tile.TileContext(nc) as tc:
        with (
            tc.tile_pool(name="const", bufs=1) as cpool,
            tc.tile_pool(name="big", bufs=1) as bigpool,
            tc.tile_pool(name="psg", bufs=3, space="PSUM") as psg,
            tc.tile_pool(name="pst", bufs=4, space="PSUM") as pst,
        ):
            ident = cpool.tile([P, P], bf16)
            make_identity(nc, ident[:])
            # prime the ACT Copy LUT early so the first scale-copy doesn't pay
            # the activation-table load mid-pipeline
            warm = cpool.tile([P, 1], f32)
            nc.vector.memset(warm[:], 0.0)
            nc.scalar.activation(warm[:], warm[:],
                                 mybir.ActivationFunctionType.Copy)

            # big transfers first: xsT on ACT, W tiles round-robin so the
            # matmul pipeline starts as early as possible; small loads follow
            # behind the first W tiles
            libload = nc.gpsimd.load_library(library_config.mlp)

            # xsT as two tiles split at the 1024-column group boundary, on
            # separate rings: groups 0-1 only depend on the low half, so the
            # matmul front opens ~2us earlier; W0 goes via the Pool ring
            # which is otherwise idle at the start
            XSPLIT = 2 * PSUM_FREE
            xsT_lo = bigpool.tile([P, XSPLIT], bf16, tag="xsTlo")
            xsT_hi = bigpool.tile([P, Ctot - XSPLIT], bf16, tag="xsThi")
            nc.scalar.dma_start(out=xsT_lo[:], in_=xsT_d[:, :XSPLIT])

            def xsT_slice(c0, c1):
                if c1 <= XSPLIT:
                    return xsT_lo[:, c0:c1]
                return xsT_hi[:, c0 - XSPLIT:c1 - XSPLIT]

            w_tiles = [bigpool.tile([P, BPW * OUT], bf16, tag=f"w{wi}",
                                    name=f"w{wi}")
                       for wi in range(NW)]

            def w_dma(eng, wi):
                eng.dma_start(
                    out=w_tiles[wi][:],
                    in_=w_d[:, wi * BPW * OUT:(wi + 1) * BPW * OUT])

            def w_slice(b):
                return w_tiles[b // BPW][:, (b % BPW) * OUT:(b % BPW + 1) * OUT]

            # SP carries S first (it gates every group's bias matmul), then
            # the other small operands and mid W tiles + xsT_hi; ACT keeps
            # only xsT_lo and two W tiles so its sequencer frees up for the
            # copy/scale chain; Pool takes the rest
            w_dma(nc.gpsimd, 0)
            s_sb = cpool.tile([nv, Ctot], fp8)
            nc.sync.dma_start(out=s_sb[:], in_=s_d.ap())
            bias_sb = cpool.tile([nv, OUT], fp8)
            nc.sync.dma_start(out=bias_sb[:], in_=bias_d.ap())
            bws_sb = cpool.tile([P, nblk], f32)
            nc.sync.dma_start(out=bws_sb[:], in_=bws_d.ap())
            w_dma(nc.scalar, 1)
            w_dma(nc.sync, 2)
            w_dma(nc.gpsimd, 3)
            if NW > 5:
                w_dma(nc.sync, 5)
            nc.sync.dma_start(out=xsT_hi[:], in_=xsT_d[:, XSPLIT:])
            if NW > 4:
                w_dma(nc.scalar, 4)
            w_engs = [nc.gpsimd, nc.sync, nc.scalar]
            for wi in range(6, NW):
                w_dma(w_engs[(wi - 6) % 3], wi)
            idx_sb = cpool.tile([P, CtotB // 16], i16)
            nc.gpsimd.dma_start(out=idx_sb[:], in_=idx_d.ap())

            # fully interleaved pipeline: per psum group do matmuls + bias,
            # copy to ysT, transpose+scale the completed 128-slot blocks, and
            # kick the A-row store chunks as soon as their blocks are scaled
            ysT = bigpool.tile([P, Ctot], bf16, tag="ysT")
            yr = bigpool.tile([P, nblk, OUT], f32, tag="yr")
            a_chunk_ends = [nblkA // 3, (2 * nblkA) // 3, nblkA]
            a_stores = []
            scale_insts = []
            next_t = 0
            next_chunk = 0
            chunk_t0 = 0
            for (col0, width, banks) in groups:
                py = psg.tile([P, PSUM_FREE], f32, tag="py")
                for j, (b, so, cb) in enumerate(banks):
                    nc.tensor.matmul(
                        out=py[:, so:so + cb],
                        lhsT=w_slice(b),
                        rhs=xsT_slice(col0 + so, col0 + so + cb),
                        start=(j == 0), stop=False, skip_group_check=True,
                    )
                nc.tensor.matmul(
                    out=py[:, 0:width], lhsT=bias_sb[:],
                    rhs=s_sb[:, col0:col0 + width],
                    start=False, stop=True, skip_group_check=True,
                )
                h = (width // 2) & ~1
                if h > 0:
                    nc.vector.tensor_copy(ysT[:, col0:col0 + h], py[:, 0:h])
                    nc.scalar.copy(ysT[:, col0 + h:col0 + width],
                                   py[:, h:width])
                else:
                    nc.vector.tensor_copy(ysT[:, col0:col0 + width],
                                          py[:, 0:width])

                done_t = (col0 + width) // P
                for t in range(next_t, done_t):
                    ptt = pst.tile([P, P], bf16, tag="ptt")
                    nc.tensor.transpose(out=ptt[:],
                                        in_=ysT[:, t * P:(t + 1) * P],
                                        identity=ident[:])
                    if t % 2 == 0 or t == nblk - 1:
                        si = nc.vector.tensor_scalar_mul(yr[:, t, :], ptt[:],
                                                         bws_sb[:, t:t + 1])
                    else:
                        si = nc.scalar.activation(
                            yr[:, t, :], ptt[:],
                            mybir.ActivationFunctionType.Copy,
                            scale=bws_sb[:, t:t + 1])
                    scale_insts.append((t, si))
                next_t = done_t
                while (next_chunk < len(a_chunk_ends)
                       and done_t >= a_chunk_ends[next_chunk]):
                    t0, t1 = chunk_t0, a_chunk_ends[next_chunk]
                    st = nc.sync.dma_start(
                        out=out_d[t0 * P:t1 * P].rearrange("(t p) o -> p t o",
                                                           p=P),
                        in_=yr[:, t0:t1, :])
                    a_stores.append(st)
                    chunk_t0 = t1
                    next_chunk += 1

            # B rows scatter-add onto their token's A row. Descriptor gen is
            # decoupled (prepare_only, runs early on the idle Pool engine);
            # one trigger fires the transfer once the A rows are in DRAM and
            # every B block is scaled.
            dma_sem = nc.alloc_semaphore("scat_dma")
            nc.gpsimd.dma_scatter_add(
                out_d.ap(),
                yr[:, nblkA:, :],
                idx_sb[:],
                CtotB, CtotB, OUT,
                prepare_only=True, sem=dma_sem,
                single_packet=CtotB <= 1024,
            )
            nc.gpsimd.trigger_dma(count=None)

    return nc


def _make_in_maps(tensor, bank_weights, bank_selections, bias, weights,
                  assign, prim, order, capsA, capsB):
    tensor = np.ascontiguousarray(tensor, dtype=np.float32)
    bank_weights = np.ascontiguousarray(bank_weights, dtype=np.float32)
    sel_all = np.asarray(bank_selections).astype(np.int64)

    CtotA = int(capsA.sum())
    CtotB = int(capsB.sum())
    Ctot = CtotA + CtotB
    nblk = Ctot // P
    NV = order.shape[1]

    offsA = np.concatenate([[0], np.cumsum(capsA)[:-1]]).astype(np.int64)
    offsB = np.concatenate([[0], np.cumsum(capsB)[:-1]]).astype(np.int64)

    x_bf = tensor.astype(BF16)
    wT_bf = np.asarray(weights, dtype=np.float32).astype(BF16) \
        .transpose(1, 0, 2)                 # [IN, NUM_BANKS, OUT]
    bias_f = np.asarray(bias, dtype=np.float32)

    in_maps = []
    slotA_all = []
    for c in range(NCORES):
        toks = assign[c]
        kprim = prim[toks]
        v_of = np.full(NUM_BANKS, -1, dtype=np.int64)
        omask = order[c] >= 0
        v_of[order[c][omask]] = np.nonzero(omask)[0]
        bA = v_of[sel_all[toks, kprim]]      # primary virtual bank per token
        bB = v_of[sel_all[toks, 1 - kprim]]  # secondary virtual bank
        assert (bA >= 0).all() and (bB >= 0).all()
        bwA = bank_weights[toks, kprim]
        bwB = bank_weights[toks, 1 - kprim]

        # per-core weight image / bias rows in virtual-slot order
        w_core = np.zeros((P, NV, OUT), dtype=BF16)
        w_core[:, omask, :] = wT_bf[:, order[c][omask], :]
        w_core = np.ascontiguousarray(w_core.reshape(P, NV * OUT))
        bias_core = np.zeros((NV, OUT), dtype=FP8)
        bias_core[omask] = bias_f[order[c][omask]].astype(FP8)

        # slot assignment: tokens fill their bank segment in order
        slotA = np.zeros(NLOC, dtype=np.int64)
        fillA = offsA.copy()
        for i in range(NLOC):
            slotA[i] = fillA[bA[i]]
            fillA[bA[i]] += 1
        slotB = np.zeros(NLOC, dtype=np.int64)
        fillB = offsB.copy()
        for i in range(NLOC):
            slotB[i] = fillB[bB[i]]
            fillB[bB[i]] += 1

        # token of each slot (-1 for pads)
        tokA = np.full(CtotA, -1, dtype=np.int64)
        tokA[slotA] = np.arange(NLOC)
        tokB = np.full(CtotB, -1, dtype=np.int64)
        tokB[slotB] = np.arange(NLOC)

        # xsT columns = x rows of the slot's token (pads zero)
        xsT = np.zeros((P, Ctot), dtype=BF16)
        mA = tokA >= 0
        xsT[:, :CtotA][:, mA] = x_bf[toks[tokA[mA]]].T
        mB = tokB >= 0
        xsT[:, CtotA:][:, mB] = x_bf[toks[tokB[mB]]].T

        # S: 1 at (bank(slot), slot) for real slots
        smat = np.zeros((NV, Ctot), dtype=FP8)
        smat[bA[tokA[mA]], np.nonzero(mA)[0]] = 1
        smat[bB[tokB[mB]], CtotA + np.nonzero(mB)[0]] = 1

        # per-slot bank weights, [p, t] grid (pads zero)
        bw_slot = np.zeros(Ctot, dtype=np.float32)
        bw_slot[slotA] = bwA
        bw_slot[CtotA + slotB] = bwB
        bws = np.ascontiguousarray(bw_slot.reshape(nblk, P).T)

        # scatter index: B slot -> its token's A slot; pads go to the dump
        # rows after the A region (unique real targets, see _build_program)
        idxB = np.zeros(CtotB, dtype=np.int16)
        idxB[mB] = slotA[tokB[mB]].astype(np.int16)
        npad = int((~mB).sum())
        idxB[~mB] = (CtotA + (np.arange(npad) % P)).astype(np.int16)

        in_maps.append({
            "xst": xsT,
            "wts": w_core,
            "smat": smat,
            "biasb": bias_core,
            "bws": bws,
            "idxb": _wrap_idx(idxB),
        })
        slotA_all.append(slotA)
    return in_maps, slotA_all


def kernel(tensor, bank_weights, bank_selections, weights, bias):
    tensor = np.asarray(tensor)
    bank_weights = np.asarray(bank_weights)
    bank_selections = np.asarray(bank_selections)
    weights = np.asarray(weights)
    bias = np.asarray(bias)

    assign, prim, order, capsA, capsB = _routing_plan(bank_selections)
    nc = _build_program(capsA, capsB)
    in_maps, slotA_all = _make_in_maps(tensor, bank_weights, bank_selections,
                                       bias, weights, assign, prim, order,
                                       capsA, capsB)

    nc.finalize()
    from concourse.bass_utils import run_bass_kernel_spmd
    try:
        res = run_bass_kernel_spmd(nc, in_maps, list(range(NCORES)))
    except Exception:
        # one retry: a previous crashed session can leave the accelerator in
        # a transient bad state that clears on the next dispatch
        import time
        time.sleep(2.0)
        res = run_bass_kernel_spmd(nc, in_maps, list(range(NCORES)))
    out = np.empty((N, OUT), dtype=np.float32)
    for c in range(NCORES):
        out[assign[c]] = res.results[c]["out"][slotA_all[c]]
    return out
